# revision 28
# baseline (speedup 1.0000x reference)
"""YOLO-v2 loss kernel for Trainium2 (8 NeuronCores, data-parallel over batch).

Decomposition:
  - GT matching (cell/anchor assignment, targets, collision/ignore logic)
    depends only on gboxes/labels -- tiny [128,8] tensors -- so it runs on the
    host, and the matched slots' 85 prediction channels are extracted on the
    host into a dense [128,85] tensor per core (43KB of the 18.4MB input).
    The device consumes (conf planes, slot logits, per-slot targets) and
    computes all loss arithmetic.
  - Dense background conf term sum(sigmoid(x)^2) over all grid positions
    reads the 5 conf planes (216KB/core), repartitioned to [128,423] (pad
    with -100 so sigmoid()==0) to use all SBUF partitions.
  - Slot channels conf/tx/ty are sign-flipped on the host so a single Exp
    pass yields e^{-x} for sigmoid terms and e^{x} for wh/cls terms.
  - Work is split across Scalar(ACT)/Vector(DVE)/Pool engines; DVE keeps the
    reciprocal + fused scalar_tensor_tensor(+accum) ops, Pool takes plain
    tensor_tensor/tensor_scalar ops.
  - Device emits per-slot partial columns [128,8]; final reduction over
    slots/cores (the all-reduce-mean step) happens on the host in f64.
"""

import numpy as np

from concourse import bass, mybir
from concourse.bass_utils import run_bass_kernel_spmd
from concourse.tile import TileContext

F32 = mybir.dt.float32
AF = mybir.ActivationFunctionType
OP = mybir.AluOpType

NC = 8                 # cores
B = 128                # batch
BL = B // NC           # images per core (16)
NGT = 8                # GT boxes per image
S = BL * NGT           # slots per core (128)
GRID = 26
HW = GRID * GRID       # 676
NCH = 85               # conf + 80 cls + 4 txywh
NANC = 5
EPS = 1e-7
CP = 128               # conf repartitioned rows
CF = 423               # conf cols (128*423 = 54144 >= 16*5*676 = 54080)
ANC = np.array([[0.05, 0.07], [0.12, 0.15], [0.25, 0.30],
                [0.45, 0.50], [0.80, 0.85]], np.float32)

# slot channel order: [conf, tx, ty, tw, th, cls0..cls79]; first 3 negated
CH_ORDER = [0, 81, 82, 83, 84] + list(range(1, 81))

# ---- merged input column layout: [slots(85) | consts(43) | onehot(80)] ----
CB = NCH               # consts base
C_GLT = CB + 2         # [2] gt lt * 26 - colrow (cell-relative)
C_GRB = CB + 4         # [2] gt rb * 26 - colrow
C_ANC26 = CB + 6       # [2] ANC[idxm] * 26
C_AREA = CB + 8        # [1] gt area * 676 + 676*eps
C_LASTW = CB + 9       # [1] last-writer mask
C_WEFF = CB + 10       # [1] effective weight (weight or -1 when ignored)
C_NOMZ = CB + 11       # [2] -(1 - txy target)  (slots tx/ty are negated)
C_TWH = CB + 13        # [2] twh target
C_OH = CB + 16         # [80] one-hot(label)
KC = CB + 96           # 181


def _host_match(gbx: np.ndarray, lbl: np.ndarray):
    """Vectorized fmatch4yolov2 mirror. gbx [B,8,4] f32, lbl [B,8] int.
    Returns consts [B*8, KC-CB] f32 and within-image gather rows [B*8] i64
    (a*676 + cell)."""
    Bn = gbx.shape[0]
    cxy = ((gbx[..., :2] + gbx[..., 2:]) * np.float32(0.5)).astype(np.float32)
    wh = (gbx[..., 2:] - gbx[..., :2]).astype(np.float32)
    c26 = cxy * np.float32(GRID)
    colrow = np.floor(c26).astype(np.float32)
    ic = colrow.astype(np.int64)
    cell = ic[..., 1] * GRID + ic[..., 0]                       # [B,8]
    inter = np.minimum(wh[:, :, None, :], ANC[None, None]).prod(-1)
    areag = wh.prod(-1)                                          # [B,8]
    iou2 = inter / (areag[..., None] + (ANC[:, 0] * ANC[:, 1])[None, None]
                    - inter + np.float32(EPS))
    mign = iou2 > 0.5                                            # [B,8,5]
    idxm = iou2.argmax(-1)                                       # [B,8]
    twh = np.log(wh / ANC[idxm]).astype(np.float32)
    weight = np.float32(2.0) - areag
    key = cell * NANC + idxm

    j_gt_i = np.triu(np.ones((NGT, NGT), bool), 1)[None]         # [1,i,j] j>i
    same_key = key[:, :, None] == key[:, None, :]
    lastw = ~(same_key & j_gt_i).any(-1)
    same_cell = cell[:, :, None] == cell[:, None, :]
    mji = np.take_along_axis(
        mign, np.broadcast_to(idxm[:, None, :], (Bn, NGT, NGT)), axis=2
    ).transpose(0, 2, 1)                                         # [B,i,j]
    ign = (same_cell & j_gt_i & mji).any(-1)
    weff = np.where(ign, np.float32(-1.0), weight)

    n = Bn * NGT
    ct = np.zeros((n, KC - CB), np.float32)
    crf = colrow.reshape(n, 2)
    ct[:, 2:4] = gbx[..., :2].reshape(n, 2) * GRID - crf
    ct[:, 4:6] = gbx[..., 2:].reshape(n, 2) * GRID - crf
    ct[:, 6:8] = ANC[idxm].reshape(n, 2) * GRID
    ct[:, 8] = areag.reshape(n) * (GRID * GRID) + GRID * GRID * EPS
    ct[:, 9] = lastw.reshape(n).astype(np.float32)
    ct[:, 10] = weff.reshape(n)
    txy = (c26 - colrow).reshape(n, 2)
    ct[:, 11:13] = txy - np.float32(1.0)                         # -(1-z)
    ct[:, 13:15] = twh.reshape(n, 2)
    oh = np.zeros((n, 80), np.float32)
    oh[np.arange(n), (lbl.reshape(n) - 1).astype(np.int64)] = 1.0
    ct[:, 16:96] = oh
    rows = (idxm * HW + cell).reshape(n)                         # within-image
    return ct, rows


def _split_multiwaits(nc: bass.Bass, k: int = 1) -> None:
    """This walrus build rejects instructions with >~2 sync waits; hoist
    extra waits onto preceding same-engine NoOps (equivalent for monotone
    sem-ge waits)."""
    for fn in nc.m.functions:
        for bb in fn.blocks:
            out = []
            for inst in bb.instructions:
                si = inst.sync_info
                waits = list(si.on_wait) if si is not None and si.on_wait else []
                if len(waits) > k:
                    for i, w in enumerate(waits[:-k]):
                        out.append(mybir.InstNoOp(
                            name=f"{inst.name}-wsplit{i}",
                            engine=inst.engine,
                            bass_nofuse=True,
                            sync_info=mybir.SyncInfo(on_wait=[w],
                                                     on_update=[]),
                        ))
                    inst.sync_info = mybir.SyncInfo(
                        on_wait=waits[-k:], on_update=list(si.on_update))
                out.append(inst)
            bb.instructions = out


def build_bass() -> bass.Bass:
    nc = bass.Bass()
    cin = nc.declare_dram_parameter("cin", [S, KC], F32, isOutput=False)
    conf = nc.declare_dram_parameter("conf", [CP, CF], mybir.dt.bfloat16,
                                     isOutput=False)
    out = nc.declare_dram_parameter("out", [S, 8], F32, isOutput=True)

    with TileContext(nc) as tc:
        with tc.tile_pool(name="sb", bufs=1) as sb:
            def tt(shape, tag, dt=F32):
                return sb.tile(shape, dt, name=tag)

            ci = tt([S, KC], "ci")
            dma_ci = nc.sync.dma_start(out=ci[:], in_=cin[:, :])
            cf = tt([CP, CF], "cf", mybir.dt.bfloat16)
            dma_cf = nc.scalar.dma_start(out=cf[:], in_=conf[:, :])

            pf = ci[:, 0:NCH]
            stack = tt([S, 8], "stack")

            # ---------------- scalar engine (Exp/Ln table set) -------------
            # u85: e^{-x} for conf/tx/ty (pre-negated), e^{x} for tw/th/cls
            u85 = tt([S, NCH], "u85")
            nc.scalar.activation(u85[:], pf, AF.Exp)
            sptxy = tt([S, 2], "sptxy")
            nc.scalar.activation(sptxy[:], u85[:, 1:3], AF.Ln, bias=1.0)
            sp80 = tt([S, 80], "sp80")
            nc.scalar.activation(sp80[:], u85[:, 5:85], AF.Ln, bias=1.0)
            spsum = tt([S, 1], "spsum")
            nc.vector.tensor_reduce(spsum[:], sp80[:], mybir.AxisListType.X,
                                    OP.add)
            # dense conf: sigmoid(x)^2 = exp(-2*softplus(-x)), 3 ACT passes
            ud = tt([CP, CF], "ud")
            nc.scalar.activation(ud[:], cf[:], AF.Exp, scale=-1.0)
            ld = tt([CP, CF], "ld")
            nc.scalar.activation(ld[:], ud[:], AF.Ln, bias=1.0)
            sd = tt([CP, CF], "sd")
            nc.scalar.activation(sd[:], ld[:], AF.Exp, scale=-2.0,
                                 accum_out=stack[:, 6:7])

            # ---------------- slot math (DVE + Pool) -----------------------
            sig3t = tt([S, 3], "sig3t")
            nc.vector.tensor_scalar_add(sig3t[:], u85[:, 0:3], 1.0)
            sig3 = tt([S, 3], "sig3")
            nc.vector.reciprocal(sig3[:], sig3t[:])

            # xlab = sum(onehot * cls logits) -- ready as soon as cin lands
            scr80 = tt([S, 80], "scr80")
            xlab = tt([S, 1], "xlab")
            nc.vector.scalar_tensor_tensor(
                out=scr80[:], in0=pf[:, 5:85], scalar=1.0,
                in1=ci[:, C_OH:C_OH + 80], op0=OP.mult, op1=OP.mult,
                accum_out=xlab[:])

            pwh = tt([S, 2], "pwh")
            nc.gpsimd.tensor_tensor(out=pwh[:], in0=u85[:, 3:5],
                                    in1=ci[:, C_ANC26:C_ANC26 + 2],
                                    op=OP.mult)
            pa = tt([S, 1], "pa")
            nc.gpsimd.tensor_tensor(out=pa[:], in0=pwh[:, 0:1],
                                    in1=pwh[:, 1:2], op=OP.mult)
            paag = tt([S, 1], "paag")
            nc.gpsimd.tensor_tensor(out=paag[:], in0=pa[:],
                                    in1=ci[:, C_AREA:C_AREA + 1], op=OP.add)
            # omzpf = -(1-z) * (-x_txy) = (1-z)*x_txy  (slots tx/ty negated)
            omzpf = tt([S, 2], "omzpf")
            nc.gpsimd.tensor_tensor(out=omzpf[:], in0=pf[:, 1:3],
                                    in1=ci[:, C_NOMZ:C_NOMZ + 2], op=OP.mult)
            dwh = tt([S, 2], "dwh")
            nc.gpsimd.tensor_tensor(out=dwh[:], in0=pf[:, 3:5],
                                    in1=ci[:, C_TWH:C_TWH + 2],
                                    op=OP.subtract)

            # cell-relative pred box corners (consts pre-translated by colrow)
            plt = tt([S, 2], "plt")
            nc.vector.scalar_tensor_tensor(
                out=plt[:], in0=pwh[:], scalar=-0.5, in1=sig3[:, 1:3],
                op0=OP.mult, op1=OP.add)
            prb = tt([S, 2], "prb")
            nc.vector.scalar_tensor_tensor(
                out=prb[:], in0=pwh[:], scalar=0.5, in1=sig3[:, 1:3],
                op0=OP.mult, op1=OP.add)
            ilt = tt([S, 2], "ilt")
            nc.vector.tensor_tensor(out=ilt[:], in0=plt[:],
                                    in1=ci[:, C_GLT:C_GLT + 2], op=OP.max)
            irb = tt([S, 2], "irb")
            nc.vector.tensor_tensor(out=irb[:], in0=prb[:],
                                    in1=ci[:, C_GRB:C_GRB + 2], op=OP.min)
            iwh = tt([S, 2], "iwh")
            nc.vector.tensor_tensor(out=iwh[:], in0=irb[:], in1=ilt[:],
                                    op=OP.subtract)
            # inter = max(iwh.x,0)*iwh.y: >0 iff both >0 (mask-exact); when
            # masked out the value is unused, and den stays positive either way
            inter = tt([S, 1], "inter")
            nc.vector.scalar_tensor_tensor(
                out=inter[:], in0=iwh[:, 0:1], scalar=0.0, in1=iwh[:, 1:2],
                op0=OP.max, op1=OP.mult)
            # mp = (inter > 0) * lastw   (den > 0 always)
            nc.vector.tensor_scalar(stack[:, 2:3], inter[:], 0.0,
                                    ci[:, C_LASTW:C_LASTW + 1],
                                    OP.is_gt, OP.mult)
            den = tt([S, 1], "den")
            nc.vector.scalar_tensor_tensor(
                out=den[:], in0=inter[:], scalar=-1.0, in1=paag[:],
                op0=OP.mult, op1=OP.add)
            # dconf = pconf - inter/den = (pconf*den - inter)/den
            num = tt([S, 1], "num")
            nc.vector.scalar_tensor_tensor(
                out=num[:], in0=den[:], scalar=sig3[:, 0:1], in1=inter[:],
                op0=OP.mult, op1=OP.subtract)
            deni = tt([S, 1], "deni")
            nc.vector.reciprocal(deni[:], den[:])
            dconf = tt([S, 1], "dconf")
            nc.vector.tensor_tensor(out=dconf[:], in0=num[:], in1=deni[:],
                                    op=OP.mult)
            # s1 = mp*dconf^2 ; s2 = mp*pconf^2 (pconf^2 on pool, early)
            nc.vector.scalar_tensor_tensor(
                out=stack[:, 0:1], in0=dconf[:], scalar=dconf[:, 0:1],
                in1=stack[:, 2:3], op0=OP.mult, op1=OP.mult)
            psq = tt([S, 1], "psq")
            nc.gpsimd.tensor_tensor(out=psq[:], in0=sig3[:, 0:1],
                                    in1=sig3[:, 0:1], op=OP.mult)
            nc.gpsimd.tensor_tensor(out=stack[:, 1:2], in0=psq[:],
                                    in1=stack[:, 2:3], op=OP.mult)

            mpw = tt([S, 1], "mpw")
            nc.gpsimd.tensor_tensor(out=mpw[:], in0=stack[:, 2:3],
                                    in1=ci[:, C_WEFF:C_WEFF + 1], op=OP.mult)
            clsn = tt([S, 1], "clsn")
            nc.gpsimd.tensor_tensor(out=clsn[:], in0=spsum[:], in1=xlab[:],
                                    op=OP.subtract)
            nc.gpsimd.tensor_tensor(out=stack[:, 3:4], in0=clsn[:],
                                    in1=stack[:, 2:3], op=OP.mult)

            bxy = tt([S, 2], "bxy")
            bcexy = tt([S, 1], "bcexy")
            nc.vector.scalar_tensor_tensor(
                out=bxy[:], in0=omzpf[:], scalar=1.0, in1=sptxy[:],
                op0=OP.mult, op1=OP.add, accum_out=bcexy[:])
            dwh2 = tt([S, 2], "dwh2")
            msewh = tt([S, 1], "msewh")
            nc.vector.scalar_tensor_tensor(
                out=dwh2[:], in0=dwh[:], scalar=1.0, in1=dwh[:],
                op0=OP.mult, op1=OP.mult, accum_out=msewh[:])
            nc.gpsimd.tensor_tensor(out=stack[:, 4:5], in0=bcexy[:],
                                    in1=mpw[:], op=OP.mult)
            nc.gpsimd.tensor_tensor(out=stack[:, 5:6], in0=msewh[:],
                                    in1=mpw[:], op=OP.mult)

            nc.sync.dma_start(out=out[:, :], in_=stack[:])

    # Hoist the two input DMA issues to the top of the main block so their
    # HBM latency overlaps the TileContext entry barrier. Their completion
    # sems move with them; consumer waits stay correct (monotone sem-ge).
    blocks = nc.m.functions[0].blocks
    main_bb = blocks[0]
    for inst in (dma_cf.ins, dma_ci.ins):
        for bb in blocks:
            if inst in bb.instructions:
                bb.instructions.remove(inst)
                break
        si = inst.sync_info
        if si is not None and si.on_wait:
            inst.sync_info = mybir.SyncInfo(on_wait=[], on_update=list(si.on_update))
        main_bb.instructions.insert(0, inst)
    _split_multiwaits(nc, k=1)
    return nc


_NC_CACHE = None
LAST_RESULTS = None


def _get_nc():
    global _NC_CACHE
    if _NC_CACHE is None:
        _NC_CACHE = build_bass()
    return _NC_CACHE


def run(pyolos, gboxes_ltrb, labels, trace=False, **spmd_kwargs):
    global LAST_RESULTS
    nc = _get_nc()
    py = np.asarray(pyolos, np.float32).reshape(B, NCH * NANC, HW)
    gbx = np.asarray(gboxes_ltrb, np.float32)
    lbl = np.asarray(labels)

    ct_all, rows = _host_match(gbx, lbl)          # [B*8, 96], [B*8]
    # slot extraction: channels for (img, anchor, cell) of each slot
    plane = (np.asarray(CH_ORDER, np.int64)[None, :] * NANC
             + (rows // HW)[:, None])             # [B*8, 85]
    pos = rows % HW
    img = np.repeat(np.arange(B), NGT)
    slots = py[img[:, None], plane, pos[:, None]]  # [B*8, 85]
    slots[:, 0:3] *= -1.0                          # conf/tx/ty sign flip
    cin_all = np.concatenate([slots, ct_all], axis=1)  # [B*8, KC]

    # pad with 0.0: sigmoid(0)^2 = 0.25 exactly; subtracted from dsq below
    import ml_dtypes
    conf_all = np.zeros((B // BL, CP * CF), ml_dtypes.bfloat16)
    cpc = py[:, 0:NANC, :].reshape(NC, BL * NANC * HW)
    conf_all[:, :BL * NANC * HW] = cpc.astype(ml_dtypes.bfloat16)

    in_maps = []
    for c in range(NC):
        in_maps.append({
            "cin": cin_all.reshape(B, NGT, KC)[c * BL:(c + 1) * BL]
                          .reshape(S, KC),
            "conf": conf_all[c].reshape(CP, CF),
        })
    res = run_bass_kernel_spmd(nc, in_maps, list(range(NC)), trace=trace,
                               **spmd_kwargs)
    LAST_RESULTS = res
    t = np.stack([r["out"] for r in res.results]).astype(np.float64)
    s1 = t[:, :, 0].sum()
    s2 = t[:, :, 1].sum()
    npos = t[:, :, 2].sum()
    s4 = t[:, :, 3].sum()
    s5 = t[:, :, 4].sum()
    s6 = t[:, :, 5].sum()
    npad = CP * CF - BL * NANC * HW
    dsq = t[:, :, 6].sum() - NC * npad * 0.25
    loss = (5.0 * s1 / B
            + (dsq - s2) / B
            + s4 / max(npos, 1.0)
            + s5 / B
            + s6 / B)
    return np.float32(loss)


def kernel(pyolos, gboxes_ltrb, labels):
    return run(pyolos, gboxes_ltrb, labels)


# revision 29
# speedup vs baseline: 1.1644x; 1.1644x over previous
"""YOLO-v2 loss kernel for Trainium2 (8 NeuronCores, data-parallel over batch).

Decomposition:
  - GT matching (cell/anchor assignment, targets, collision/ignore logic)
    depends only on gboxes/labels -- tiny [128,8] tensors -- so it runs on the
    host, and the matched slots' 85 prediction channels are extracted on the
    host into a dense [128,85] tensor per core (43KB of the 18.4MB input).
    The device consumes (conf planes, slot logits, per-slot targets) and
    computes all loss arithmetic.
  - Dense background conf term sum(sigmoid(x)^2) over all grid positions
    reads the 5 conf planes (216KB/core), repartitioned to [128,423] (pad
    with -100 so sigmoid()==0) to use all SBUF partitions.
  - Slot channels conf/tx/ty are sign-flipped on the host so a single Exp
    pass yields e^{-x} for sigmoid terms and e^{x} for wh/cls terms.
  - Work is split across Scalar(ACT)/Vector(DVE)/Pool engines; DVE keeps the
    reciprocal + fused scalar_tensor_tensor(+accum) ops, Pool takes plain
    tensor_tensor/tensor_scalar ops.
  - Device emits per-slot partial columns [128,8]; final reduction over
    slots/cores (the all-reduce-mean step) happens on the host in f64.
"""

import numpy as np

from concourse import bass, mybir
from concourse.bass_utils import run_bass_kernel_spmd
from concourse.tile import TileContext

F32 = mybir.dt.float32
AF = mybir.ActivationFunctionType
OP = mybir.AluOpType

NC = 8                 # cores
B = 128                # batch
BL = B // NC           # images per core (16)
NGT = 8                # GT boxes per image
S = BL * NGT           # slots per core (128)
GRID = 26
HW = GRID * GRID       # 676
NCH = 85               # conf + 80 cls + 4 txywh
NANC = 5
EPS = 1e-7
CP = 128               # conf repartitioned rows
CF = 423               # conf cols (128*423 = 54144 >= 16*5*676 = 54080)
ANC = np.array([[0.05, 0.07], [0.12, 0.15], [0.25, 0.30],
                [0.45, 0.50], [0.80, 0.85]], np.float32)

# slot channel order: [conf, tx, ty, tw, th, cls0..cls79]; first 3 negated
CH_ORDER = [0, 81, 82, 83, 84] + list(range(1, 81))

# ---- merged input column layout: [slots(85) | consts(43) | onehot(80)] ----
CB = NCH               # consts base
C_GLT = CB + 2         # [2] gt lt * 26 - colrow (cell-relative)
C_GRB = CB + 4         # [2] gt rb * 26 - colrow
C_ANC26 = CB + 6       # [2] ANC[idxm] * 26
C_AREA = CB + 8        # [1] gt area * 676 + 676*eps
C_LASTW = CB + 9       # [1] last-writer mask
C_WEFF = CB + 10       # [1] effective weight (weight or -1 when ignored)
C_NOMZ = CB + 11       # [2] -(1 - txy target)  (slots tx/ty are negated)
C_TWH = CB + 13        # [2] twh target
C_OH = CB + 16         # [80] one-hot(label)
KC = CB + 96           # 181


def _host_match(gbx: np.ndarray, lbl: np.ndarray):
    """Vectorized fmatch4yolov2 mirror. gbx [B,8,4] f32, lbl [B,8] int.
    Returns consts [B*8, KC-CB] f32 and within-image gather rows [B*8] i64
    (a*676 + cell)."""
    Bn = gbx.shape[0]
    cxy = ((gbx[..., :2] + gbx[..., 2:]) * np.float32(0.5)).astype(np.float32)
    wh = (gbx[..., 2:] - gbx[..., :2]).astype(np.float32)
    c26 = cxy * np.float32(GRID)
    colrow = np.floor(c26).astype(np.float32)
    ic = colrow.astype(np.int64)
    cell = ic[..., 1] * GRID + ic[..., 0]                       # [B,8]
    inter = np.minimum(wh[:, :, None, :], ANC[None, None]).prod(-1)
    areag = wh.prod(-1)                                          # [B,8]
    iou2 = inter / (areag[..., None] + (ANC[:, 0] * ANC[:, 1])[None, None]
                    - inter + np.float32(EPS))
    mign = iou2 > 0.5                                            # [B,8,5]
    idxm = iou2.argmax(-1)                                       # [B,8]
    twh = np.log(wh / ANC[idxm]).astype(np.float32)
    weight = np.float32(2.0) - areag
    key = cell * NANC + idxm

    j_gt_i = np.triu(np.ones((NGT, NGT), bool), 1)[None]         # [1,i,j] j>i
    same_key = key[:, :, None] == key[:, None, :]
    lastw = ~(same_key & j_gt_i).any(-1)
    same_cell = cell[:, :, None] == cell[:, None, :]
    mji = np.take_along_axis(
        mign, np.broadcast_to(idxm[:, None, :], (Bn, NGT, NGT)), axis=2
    ).transpose(0, 2, 1)                                         # [B,i,j]
    ign = (same_cell & j_gt_i & mji).any(-1)
    weff = np.where(ign, np.float32(-1.0), weight)

    n = Bn * NGT
    ct = np.zeros((n, KC - CB), np.float32)
    crf = colrow.reshape(n, 2)
    ct[:, 2:4] = gbx[..., :2].reshape(n, 2) * GRID - crf
    ct[:, 4:6] = gbx[..., 2:].reshape(n, 2) * GRID - crf
    ct[:, 6:8] = ANC[idxm].reshape(n, 2) * GRID
    ct[:, 8] = areag.reshape(n) * (GRID * GRID) + GRID * GRID * EPS
    ct[:, 9] = lastw.reshape(n).astype(np.float32)
    ct[:, 10] = weff.reshape(n)
    txy = (c26 - colrow).reshape(n, 2)
    ct[:, 11:13] = txy - np.float32(1.0)                         # -(1-z)
    ct[:, 13:15] = twh.reshape(n, 2)
    oh = np.zeros((n, 80), np.float32)
    oh[np.arange(n), (lbl.reshape(n) - 1).astype(np.int64)] = 1.0
    ct[:, 16:96] = oh
    rows = (idxm * HW + cell).reshape(n)                         # within-image
    return ct, rows


def _split_multiwaits(nc: bass.Bass, k: int = 1) -> None:
    """This walrus build rejects instructions with >~2 sync waits; hoist
    extra waits onto preceding same-engine NoOps (equivalent for monotone
    sem-ge waits)."""
    for fn in nc.m.functions:
        for bb in fn.blocks:
            out = []
            for inst in bb.instructions:
                si = inst.sync_info
                waits = list(si.on_wait) if si is not None and si.on_wait else []
                if len(waits) > k:
                    for i, w in enumerate(waits[:-k]):
                        out.append(mybir.InstNoOp(
                            name=f"{inst.name}-wsplit{i}",
                            engine=inst.engine,
                            bass_nofuse=True,
                            sync_info=mybir.SyncInfo(on_wait=[w],
                                                     on_update=[]),
                        ))
                    inst.sync_info = mybir.SyncInfo(
                        on_wait=waits[-k:], on_update=list(si.on_update))
                out.append(inst)
            bb.instructions = out


def build_bass() -> bass.Bass:
    nc = bass.Bass()
    cin = nc.declare_dram_parameter("cin", [S, KC], F32, isOutput=False)
    conf = nc.declare_dram_parameter("conf", [CP, CF], mybir.dt.bfloat16,
                                     isOutput=False)
    out = nc.declare_dram_parameter("out", [S, 8], F32, isOutput=True)

    with TileContext(nc) as tc:
        with tc.tile_pool(name="sb", bufs=1) as sb:
            def tt(shape, tag, dt=F32):
                return sb.tile(shape, dt, name=tag)

            ci = tt([S, KC], "ci")
            dma_ci = nc.sync.dma_start(out=ci[:], in_=cin[:, :])
            cf = tt([CP, CF], "cf", mybir.dt.bfloat16)
            dma_cf = nc.scalar.dma_start(out=cf[:], in_=conf[:, :])

            pf = ci[:, 0:NCH]
            stack = tt([S, 8], "stack")

            # ---------------- scalar engine (Exp/Ln table set) -------------
            # u85: e^{-x} for conf/tx/ty (pre-negated), e^{x} for tw/th/cls
            u85 = tt([S, NCH], "u85")
            nc.scalar.activation(u85[:], pf, AF.Exp)
            sptxy = tt([S, 2], "sptxy")
            nc.scalar.activation(sptxy[:], u85[:, 1:3], AF.Ln, bias=1.0)
            sp80 = tt([S, 80], "sp80")
            nc.scalar.activation(sp80[:], u85[:, 5:85], AF.Ln, bias=1.0)
            spsum = tt([S, 1], "spsum")
            nc.vector.tensor_reduce(spsum[:], sp80[:], mybir.AxisListType.X,
                                    OP.add)
            # dense conf: sigmoid(x)^2 = exp(-2*softplus(-x)), 3 ACT passes
            ud = tt([CP, CF], "ud")
            nc.scalar.activation(ud[:], cf[:], AF.Exp, scale=-1.0)
            ld = tt([CP, CF], "ld")
            nc.scalar.activation(ld[:], ud[:], AF.Ln, bias=1.0)
            sd = tt([CP, CF], "sd")
            nc.scalar.activation(sd[:], ld[:], AF.Exp, scale=-2.0,
                                 accum_out=stack[:, 6:7])

            # ---------------- slot math (DVE + Pool) -----------------------
            sig3t = tt([S, 3], "sig3t")
            nc.vector.tensor_scalar_add(sig3t[:], u85[:, 0:3], 1.0)
            sig3 = tt([S, 3], "sig3")
            nc.vector.reciprocal(sig3[:], sig3t[:])

            # xlab = sum(onehot * cls logits) -- ready as soon as cin lands
            scr80 = tt([S, 80], "scr80")
            xlab = tt([S, 1], "xlab")
            nc.vector.scalar_tensor_tensor(
                out=scr80[:], in0=pf[:, 5:85], scalar=1.0,
                in1=ci[:, C_OH:C_OH + 80], op0=OP.mult, op1=OP.mult,
                accum_out=xlab[:])

            pwh = tt([S, 2], "pwh")
            nc.gpsimd.tensor_tensor(out=pwh[:], in0=u85[:, 3:5],
                                    in1=ci[:, C_ANC26:C_ANC26 + 2],
                                    op=OP.mult)
            pa = tt([S, 1], "pa")
            nc.gpsimd.tensor_tensor(out=pa[:], in0=pwh[:, 0:1],
                                    in1=pwh[:, 1:2], op=OP.mult)
            paag = tt([S, 1], "paag")
            nc.gpsimd.tensor_tensor(out=paag[:], in0=pa[:],
                                    in1=ci[:, C_AREA:C_AREA + 1], op=OP.add)
            # omzpf = -(1-z) * (-x_txy) = (1-z)*x_txy  (slots tx/ty negated)
            omzpf = tt([S, 2], "omzpf")
            nc.gpsimd.tensor_tensor(out=omzpf[:], in0=pf[:, 1:3],
                                    in1=ci[:, C_NOMZ:C_NOMZ + 2], op=OP.mult)
            dwh = tt([S, 2], "dwh")
            nc.gpsimd.tensor_tensor(out=dwh[:], in0=pf[:, 3:5],
                                    in1=ci[:, C_TWH:C_TWH + 2],
                                    op=OP.subtract)

            # cell-relative pred box corners (consts pre-translated by colrow)
            plt = tt([S, 2], "plt")
            nc.vector.scalar_tensor_tensor(
                out=plt[:], in0=pwh[:], scalar=-0.5, in1=sig3[:, 1:3],
                op0=OP.mult, op1=OP.add)
            prb = tt([S, 2], "prb")
            nc.vector.scalar_tensor_tensor(
                out=prb[:], in0=pwh[:], scalar=0.5, in1=sig3[:, 1:3],
                op0=OP.mult, op1=OP.add)
            ilt = tt([S, 2], "ilt")
            nc.vector.tensor_tensor(out=ilt[:], in0=plt[:],
                                    in1=ci[:, C_GLT:C_GLT + 2], op=OP.max)
            irb = tt([S, 2], "irb")
            nc.vector.tensor_tensor(out=irb[:], in0=prb[:],
                                    in1=ci[:, C_GRB:C_GRB + 2], op=OP.min)
            iwh = tt([S, 2], "iwh")
            nc.vector.tensor_tensor(out=iwh[:], in0=irb[:], in1=ilt[:],
                                    op=OP.subtract)
            # inter = max(iwh.x,0)*iwh.y: >0 iff both >0 (mask-exact); when
            # masked out the value is unused, and den stays positive either way
            inter = tt([S, 1], "inter")
            nc.vector.scalar_tensor_tensor(
                out=inter[:], in0=iwh[:, 0:1], scalar=0.0, in1=iwh[:, 1:2],
                op0=OP.max, op1=OP.mult)
            # mp = (inter > 0) * lastw   (den > 0 always)
            nc.vector.tensor_scalar(stack[:, 2:3], inter[:], 0.0,
                                    ci[:, C_LASTW:C_LASTW + 1],
                                    OP.is_gt, OP.mult)
            den = tt([S, 1], "den")
            nc.vector.scalar_tensor_tensor(
                out=den[:], in0=inter[:], scalar=-1.0, in1=paag[:],
                op0=OP.mult, op1=OP.add)
            # dconf = pconf - inter/den = (pconf*den - inter)/den
            num = tt([S, 1], "num")
            nc.vector.scalar_tensor_tensor(
                out=num[:], in0=den[:], scalar=sig3[:, 0:1], in1=inter[:],
                op0=OP.mult, op1=OP.subtract)
            deni = tt([S, 1], "deni")
            nc.vector.reciprocal(deni[:], den[:])
            dconf = tt([S, 1], "dconf")
            nc.vector.tensor_tensor(out=dconf[:], in0=num[:], in1=deni[:],
                                    op=OP.mult)
            # s1 = mp*dconf^2 ; s2 = mp*pconf^2 (pconf^2 on pool, early)
            nc.vector.scalar_tensor_tensor(
                out=stack[:, 0:1], in0=dconf[:], scalar=dconf[:, 0:1],
                in1=stack[:, 2:3], op0=OP.mult, op1=OP.mult)
            psq = tt([S, 1], "psq")
            nc.gpsimd.tensor_tensor(out=psq[:], in0=sig3[:, 0:1],
                                    in1=sig3[:, 0:1], op=OP.mult)
            nc.gpsimd.tensor_tensor(out=stack[:, 1:2], in0=psq[:],
                                    in1=stack[:, 2:3], op=OP.mult)

            mpw = tt([S, 1], "mpw")
            nc.gpsimd.tensor_tensor(out=mpw[:], in0=stack[:, 2:3],
                                    in1=ci[:, C_WEFF:C_WEFF + 1], op=OP.mult)
            clsn = tt([S, 1], "clsn")
            nc.gpsimd.tensor_tensor(out=clsn[:], in0=spsum[:], in1=xlab[:],
                                    op=OP.subtract)
            nc.gpsimd.tensor_tensor(out=stack[:, 3:4], in0=clsn[:],
                                    in1=stack[:, 2:3], op=OP.mult)

            bxy = tt([S, 2], "bxy")
            bcexy = tt([S, 1], "bcexy")
            nc.vector.scalar_tensor_tensor(
                out=bxy[:], in0=omzpf[:], scalar=1.0, in1=sptxy[:],
                op0=OP.mult, op1=OP.add, accum_out=bcexy[:])
            dwh2 = tt([S, 2], "dwh2")
            msewh = tt([S, 1], "msewh")
            nc.vector.scalar_tensor_tensor(
                out=dwh2[:], in0=dwh[:], scalar=1.0, in1=dwh[:],
                op0=OP.mult, op1=OP.mult, accum_out=msewh[:])
            nc.gpsimd.tensor_tensor(out=stack[:, 4:5], in0=bcexy[:],
                                    in1=mpw[:], op=OP.mult)
            nc.gpsimd.tensor_tensor(out=stack[:, 5:6], in0=msewh[:],
                                    in1=mpw[:], op=OP.mult)

            nc.sync.dma_start(out=out[:, :], in_=stack[:])

    # Hoist the two input DMA issues and the ACT table load to the top of
    # the main block so their latency overlaps the TileContext entry
    # barrier. Completion sems move with them; consumer waits stay correct
    # (monotone sem-ge).
    blocks = nc.m.functions[0].blocks
    main_bb = blocks[0]
    hoist = [dma_cf.ins, dma_ci.ins]
    for bb in blocks:
        for inst in list(bb.instructions):
            if inst.name.endswith("-PWP"):
                hoist.append(inst)
    for inst in reversed(hoist):
        for bb in blocks:
            if inst in bb.instructions:
                bb.instructions.remove(inst)
                break
        si = inst.sync_info
        if si is not None and si.on_wait:
            inst.sync_info = mybir.SyncInfo(on_wait=[], on_update=list(si.on_update))
        main_bb.instructions.insert(0, inst)
    _split_multiwaits(nc, k=1)
    return nc


_NC_CACHE = None
LAST_RESULTS = None


def _get_nc():
    global _NC_CACHE
    if _NC_CACHE is None:
        _NC_CACHE = build_bass()
    return _NC_CACHE


def run(pyolos, gboxes_ltrb, labels, trace=False, **spmd_kwargs):
    global LAST_RESULTS
    nc = _get_nc()
    py = np.asarray(pyolos, np.float32).reshape(B, NCH * NANC, HW)
    gbx = np.asarray(gboxes_ltrb, np.float32)
    lbl = np.asarray(labels)

    ct_all, rows = _host_match(gbx, lbl)          # [B*8, 96], [B*8]
    # slot extraction: channels for (img, anchor, cell) of each slot
    plane = (np.asarray(CH_ORDER, np.int64)[None, :] * NANC
             + (rows // HW)[:, None])             # [B*8, 85]
    pos = rows % HW
    img = np.repeat(np.arange(B), NGT)
    slots = py[img[:, None], plane, pos[:, None]]  # [B*8, 85]
    slots[:, 0:3] *= -1.0                          # conf/tx/ty sign flip
    cin_all = np.concatenate([slots, ct_all], axis=1)  # [B*8, KC]

    # pad with 0.0: sigmoid(0)^2 = 0.25 exactly; subtracted from dsq below
    import ml_dtypes
    conf_all = np.zeros((B // BL, CP * CF), ml_dtypes.bfloat16)
    cpc = py[:, 0:NANC, :].reshape(NC, BL * NANC * HW)
    conf_all[:, :BL * NANC * HW] = cpc.astype(ml_dtypes.bfloat16)

    in_maps = []
    for c in range(NC):
        in_maps.append({
            "cin": cin_all.reshape(B, NGT, KC)[c * BL:(c + 1) * BL]
                          .reshape(S, KC),
            "conf": conf_all[c].reshape(CP, CF),
        })
    res = run_bass_kernel_spmd(nc, in_maps, list(range(NC)), trace=trace,
                               **spmd_kwargs)
    LAST_RESULTS = res
    t = np.stack([r["out"] for r in res.results]).astype(np.float64)
    s1 = t[:, :, 0].sum()
    s2 = t[:, :, 1].sum()
    npos = t[:, :, 2].sum()
    s4 = t[:, :, 3].sum()
    s5 = t[:, :, 4].sum()
    s6 = t[:, :, 5].sum()
    npad = CP * CF - BL * NANC * HW
    dsq = t[:, :, 6].sum() - NC * npad * 0.25
    loss = (5.0 * s1 / B
            + (dsq - s2) / B
            + s4 / max(npos, 1.0)
            + s5 / B
            + s6 / B)
    return np.float32(loss)


def kernel(pyolos, gboxes_ltrb, labels):
    return run(pyolos, gboxes_ltrb, labels)


# revision 32
# speedup vs baseline: 1.1973x; 1.0283x over previous
"""YOLO-v2 loss kernel for Trainium2 (8 NeuronCores, data-parallel over batch).

Decomposition:
  - GT matching (cell/anchor assignment, targets, collision/ignore logic)
    depends only on gboxes/labels -- tiny [128,8] tensors -- so it runs on the
    host, and the matched slots' 85 prediction channels are extracted on the
    host into a dense [128,85] tensor per core (43KB of the 18.4MB input).
    The device consumes (conf planes, slot logits, per-slot targets) and
    computes all loss arithmetic.
  - Dense background conf term sum(sigmoid(x)^2) over all grid positions
    reads the 5 conf planes (216KB/core), repartitioned to [128,423] (pad
    with -100 so sigmoid()==0) to use all SBUF partitions.
  - Slot channels conf/tx/ty are sign-flipped on the host so a single Exp
    pass yields e^{-x} for sigmoid terms and e^{x} for wh/cls terms.
  - Work is split across Scalar(ACT)/Vector(DVE)/Pool engines; DVE keeps the
    reciprocal + fused scalar_tensor_tensor(+accum) ops, Pool takes plain
    tensor_tensor/tensor_scalar ops.
  - Device emits per-slot partial columns [128,8]; final reduction over
    slots/cores (the all-reduce-mean step) happens on the host in f64.
"""

import numpy as np

from concourse import bass, mybir
from concourse.bass_utils import run_bass_kernel_spmd
from concourse.tile import TileContext

F32 = mybir.dt.float32
AF = mybir.ActivationFunctionType
OP = mybir.AluOpType

NC = 8                 # cores
B = 128                # batch
BL = B // NC           # images per core (16)
NGT = 8                # GT boxes per image
S = BL * NGT           # slots per core (128)
GRID = 26
HW = GRID * GRID       # 676
NCH = 85               # conf + 80 cls + 4 txywh
NANC = 5
EPS = 1e-7
CP = 128               # conf repartitioned rows
CF = 423               # conf cols (128*423 = 54144 >= 16*5*676 = 54080)
ANC = np.array([[0.05, 0.07], [0.12, 0.15], [0.25, 0.30],
                [0.45, 0.50], [0.80, 0.85]], np.float32)

# slot channel order: [conf, tx, ty, tw, th, cls0..cls79]; first 3 negated
CH_ORDER = [0, 81, 82, 83, 84] + list(range(1, 81))

# ---- merged input column layout: [slots(85) | consts(43) | onehot(80)] ----
CB = NCH               # consts base
C_GLT = CB + 2         # [2] gt lt * 26 - colrow (cell-relative)
C_GRB = CB + 4         # [2] gt rb * 26 - colrow
C_ANC26 = CB + 6       # [2] ANC[idxm] * 26
C_AREA = CB + 8        # [1] gt area * 676 + 676*eps
C_LASTW = CB + 9       # [1] last-writer mask
C_WEFF = CB + 10       # [1] effective weight (weight or -1 when ignored)
C_NOMZ = CB + 11       # [2] -(1 - txy target)  (slots tx/ty are negated)
C_TWH = CB + 13        # [2] twh target
C_OH = CB + 16         # [80] one-hot(label)
KC = CB + 96           # 181


def _host_match(gbx: np.ndarray, lbl: np.ndarray):
    """Vectorized fmatch4yolov2 mirror. gbx [B,8,4] f32, lbl [B,8] int.
    Returns consts [B*8, KC-CB] f32 and within-image gather rows [B*8] i64
    (a*676 + cell)."""
    Bn = gbx.shape[0]
    cxy = ((gbx[..., :2] + gbx[..., 2:]) * np.float32(0.5)).astype(np.float32)
    wh = (gbx[..., 2:] - gbx[..., :2]).astype(np.float32)
    c26 = cxy * np.float32(GRID)
    colrow = np.floor(c26).astype(np.float32)
    ic = colrow.astype(np.int64)
    cell = ic[..., 1] * GRID + ic[..., 0]                       # [B,8]
    inter = np.minimum(wh[:, :, None, :], ANC[None, None]).prod(-1)
    areag = wh.prod(-1)                                          # [B,8]
    iou2 = inter / (areag[..., None] + (ANC[:, 0] * ANC[:, 1])[None, None]
                    - inter + np.float32(EPS))
    mign = iou2 > 0.5                                            # [B,8,5]
    idxm = iou2.argmax(-1)                                       # [B,8]
    twh = np.log(wh / ANC[idxm]).astype(np.float32)
    weight = np.float32(2.0) - areag
    key = cell * NANC + idxm

    j_gt_i = np.triu(np.ones((NGT, NGT), bool), 1)[None]         # [1,i,j] j>i
    same_key = key[:, :, None] == key[:, None, :]
    lastw = ~(same_key & j_gt_i).any(-1)
    same_cell = cell[:, :, None] == cell[:, None, :]
    mji = np.take_along_axis(
        mign, np.broadcast_to(idxm[:, None, :], (Bn, NGT, NGT)), axis=2
    ).transpose(0, 2, 1)                                         # [B,i,j]
    ign = (same_cell & j_gt_i & mji).any(-1)
    weff = np.where(ign, np.float32(-1.0), weight)

    n = Bn * NGT
    ct = np.zeros((n, KC - CB), np.float32)
    crf = colrow.reshape(n, 2)
    ct[:, 2:4] = gbx[..., :2].reshape(n, 2) * GRID - crf
    ct[:, 4:6] = gbx[..., 2:].reshape(n, 2) * GRID - crf
    ct[:, 6:8] = ANC[idxm].reshape(n, 2) * GRID
    ct[:, 8] = areag.reshape(n) * (GRID * GRID) + GRID * GRID * EPS
    ct[:, 9] = lastw.reshape(n).astype(np.float32)
    ct[:, 10] = weff.reshape(n)
    txy = (c26 - colrow).reshape(n, 2)
    ct[:, 11:13] = txy - np.float32(1.0)                         # -(1-z)
    ct[:, 13:15] = twh.reshape(n, 2)
    oh = np.zeros((n, 80), np.float32)
    oh[np.arange(n), (lbl.reshape(n) - 1).astype(np.int64)] = 1.0
    ct[:, 16:96] = oh
    rows = (idxm * HW + cell).reshape(n)                         # within-image
    return ct, rows


def _split_multiwaits(nc: bass.Bass, k: int = 1) -> None:
    """This walrus build rejects instructions with >~2 sync waits; hoist
    extra waits onto preceding same-engine NoOps (equivalent for monotone
    sem-ge waits)."""
    for fn in nc.m.functions:
        for bb in fn.blocks:
            out = []
            for inst in bb.instructions:
                si = inst.sync_info
                waits = list(si.on_wait) if si is not None and si.on_wait else []
                if len(waits) > k:
                    for i, w in enumerate(waits[:-k]):
                        out.append(mybir.InstNoOp(
                            name=f"{inst.name}-wsplit{i}",
                            engine=inst.engine,
                            bass_nofuse=True,
                            sync_info=mybir.SyncInfo(on_wait=[w],
                                                     on_update=[]),
                        ))
                    inst.sync_info = mybir.SyncInfo(
                        on_wait=waits[-k:], on_update=list(si.on_update))
                out.append(inst)
            bb.instructions = out


def build_bass() -> bass.Bass:
    nc = bass.Bass()
    cin = nc.declare_dram_parameter("cin", [S, KC], F32, isOutput=False)
    conf = nc.declare_dram_parameter("conf", [CP, CF], mybir.dt.bfloat16,
                                     isOutput=False)
    out = nc.declare_dram_parameter("out", [S, 8], F32, isOutput=True)

    with TileContext(nc) as tc:
        with tc.tile_pool(name="sb", bufs=1) as sb:
            def tt(shape, tag, dt=F32):
                return sb.tile(shape, dt, name=tag)

            ci = tt([S, KC], "ci")
            dma_ci = nc.sync.dma_start(out=ci[:], in_=cin[:, :])
            cf = tt([CP, CF], "cf", mybir.dt.bfloat16)
            dma_cf = nc.scalar.dma_start(out=cf[:], in_=conf[:, :])

            pf = ci[:, 0:NCH]
            stack = tt([S, 8], "stack")

            # ---------------- scalar engine (Exp/Ln table set) -------------
            # u85: e^{-x} for conf/tx/ty (pre-negated), e^{x} for tw/th/cls
            u85 = tt([S, NCH], "u85")
            nc.scalar.activation(u85[:], pf, AF.Exp)
            # one Ln pass: cols 1:3 = softplus(-txy), 5:85 = softplus(cls);
            # cols 0,3,4 are unused junk
            l85 = tt([S, NCH], "l85")
            nc.scalar.activation(l85[:], u85[:], AF.Ln, bias=1.0)
            sptxy = l85[:, 1:3]
            spsum = tt([S, 1], "spsum")
            nc.vector.tensor_reduce(spsum[:], l85[:, 5:85],
                                    mybir.AxisListType.X, OP.add)
            # dense conf: sigmoid(x)^2 = exp(-2*softplus(-x)), 3 ACT passes
            ud = tt([CP, CF], "ud")
            nc.scalar.activation(ud[:], cf[:], AF.Exp, scale=-1.0)
            ld = tt([CP, CF], "ld")
            nc.scalar.activation(ld[:], ud[:], AF.Ln, bias=1.0)
            sd = tt([CP, CF], "sd")
            nc.scalar.activation(sd[:], ld[:], AF.Exp, scale=-2.0,
                                 accum_out=stack[:, 6:7])

            # ---------------- slot math (DVE + Pool) -----------------------
            sig3t = tt([S, 3], "sig3t")
            nc.vector.tensor_scalar_add(sig3t[:], u85[:, 0:3], 1.0)
            sig3 = tt([S, 3], "sig3")
            nc.vector.reciprocal(sig3[:], sig3t[:])

            # xlab = sum(onehot * cls logits) -- ready as soon as cin lands
            scr80 = tt([S, 80], "scr80")
            xlab = tt([S, 1], "xlab")
            nc.vector.scalar_tensor_tensor(
                out=scr80[:], in0=pf[:, 5:85], scalar=1.0,
                in1=ci[:, C_OH:C_OH + 80], op0=OP.mult, op1=OP.mult,
                accum_out=xlab[:])

            pwh = tt([S, 2], "pwh")
            nc.gpsimd.tensor_tensor(out=pwh[:], in0=u85[:, 3:5],
                                    in1=ci[:, C_ANC26:C_ANC26 + 2],
                                    op=OP.mult)
            pa = tt([S, 1], "pa")
            nc.gpsimd.tensor_tensor(out=pa[:], in0=pwh[:, 0:1],
                                    in1=pwh[:, 1:2], op=OP.mult)
            paag = tt([S, 1], "paag")
            nc.gpsimd.tensor_tensor(out=paag[:], in0=pa[:],
                                    in1=ci[:, C_AREA:C_AREA + 1], op=OP.add)
            # omzpf = -(1-z) * (-x_txy) = (1-z)*x_txy  (slots tx/ty negated)
            omzpf = tt([S, 2], "omzpf")
            nc.gpsimd.tensor_tensor(out=omzpf[:], in0=pf[:, 1:3],
                                    in1=ci[:, C_NOMZ:C_NOMZ + 2], op=OP.mult)
            dwh = tt([S, 2], "dwh")
            nc.gpsimd.tensor_tensor(out=dwh[:], in0=pf[:, 3:5],
                                    in1=ci[:, C_TWH:C_TWH + 2],
                                    op=OP.subtract)

            # cell-relative pred box corners (consts pre-translated by colrow)
            plt = tt([S, 2], "plt")
            nc.vector.scalar_tensor_tensor(
                out=plt[:], in0=pwh[:], scalar=-0.5, in1=sig3[:, 1:3],
                op0=OP.mult, op1=OP.add)
            prb = tt([S, 2], "prb")
            nc.vector.scalar_tensor_tensor(
                out=prb[:], in0=pwh[:], scalar=0.5, in1=sig3[:, 1:3],
                op0=OP.mult, op1=OP.add)
            ilt = tt([S, 2], "ilt")
            nc.vector.tensor_tensor(out=ilt[:], in0=plt[:],
                                    in1=ci[:, C_GLT:C_GLT + 2], op=OP.max)
            irb = tt([S, 2], "irb")
            nc.vector.tensor_tensor(out=irb[:], in0=prb[:],
                                    in1=ci[:, C_GRB:C_GRB + 2], op=OP.min)
            iwh = tt([S, 2], "iwh")
            nc.vector.tensor_tensor(out=iwh[:], in0=irb[:], in1=ilt[:],
                                    op=OP.subtract)
            # inter = max(iwh.x,0)*iwh.y: >0 iff both >0 (mask-exact); when
            # masked out the value is unused, and den stays positive either way
            inter = tt([S, 1], "inter")
            nc.vector.scalar_tensor_tensor(
                out=inter[:], in0=iwh[:, 0:1], scalar=0.0, in1=iwh[:, 1:2],
                op0=OP.max, op1=OP.mult)
            # mp = (inter > 0) * lastw   (den > 0 always)
            nc.vector.tensor_scalar(stack[:, 2:3], inter[:], 0.0,
                                    ci[:, C_LASTW:C_LASTW + 1],
                                    OP.is_gt, OP.mult)
            den = tt([S, 1], "den")
            nc.vector.scalar_tensor_tensor(
                out=den[:], in0=inter[:], scalar=-1.0, in1=paag[:],
                op0=OP.mult, op1=OP.add)
            # dconf = pconf - inter/den = (pconf*den - inter)/den
            num = tt([S, 1], "num")
            nc.vector.scalar_tensor_tensor(
                out=num[:], in0=den[:], scalar=sig3[:, 0:1], in1=inter[:],
                op0=OP.mult, op1=OP.subtract)
            deni = tt([S, 1], "deni")
            nc.vector.reciprocal(deni[:], den[:])
            dconf = tt([S, 1], "dconf")
            nc.vector.tensor_tensor(out=dconf[:], in0=num[:], in1=deni[:],
                                    op=OP.mult)
            # s1 = mp*dconf^2 ; s2 = mp*pconf^2 (pconf^2 on pool, early)
            nc.vector.scalar_tensor_tensor(
                out=stack[:, 0:1], in0=dconf[:], scalar=dconf[:, 0:1],
                in1=stack[:, 2:3], op0=OP.mult, op1=OP.mult)
            psq = tt([S, 1], "psq")
            nc.gpsimd.tensor_tensor(out=psq[:], in0=sig3[:, 0:1],
                                    in1=sig3[:, 0:1], op=OP.mult)
            nc.gpsimd.tensor_tensor(out=stack[:, 1:2], in0=psq[:],
                                    in1=stack[:, 2:3], op=OP.mult)

            mpw = tt([S, 1], "mpw")
            nc.gpsimd.tensor_tensor(out=mpw[:], in0=stack[:, 2:3],
                                    in1=ci[:, C_WEFF:C_WEFF + 1], op=OP.mult)
            clsn = tt([S, 1], "clsn")
            nc.gpsimd.tensor_tensor(out=clsn[:], in0=spsum[:], in1=xlab[:],
                                    op=OP.subtract)
            nc.gpsimd.tensor_tensor(out=stack[:, 3:4], in0=clsn[:],
                                    in1=stack[:, 2:3], op=OP.mult)

            bxy = tt([S, 2], "bxy")
            bcexy = tt([S, 1], "bcexy")
            nc.vector.scalar_tensor_tensor(
                out=bxy[:], in0=omzpf[:], scalar=1.0, in1=sptxy,
                op0=OP.mult, op1=OP.add, accum_out=bcexy[:])
            dwh2 = tt([S, 2], "dwh2")
            msewh = tt([S, 1], "msewh")
            nc.vector.scalar_tensor_tensor(
                out=dwh2[:], in0=dwh[:], scalar=1.0, in1=dwh[:],
                op0=OP.mult, op1=OP.mult, accum_out=msewh[:])
            nc.gpsimd.tensor_tensor(out=stack[:, 4:5], in0=bcexy[:],
                                    in1=mpw[:], op=OP.mult)
            nc.gpsimd.tensor_tensor(out=stack[:, 5:6], in0=msewh[:],
                                    in1=mpw[:], op=OP.mult)

            nc.sync.dma_start(out=out[:, :], in_=stack[:])

    # Hoist the two input DMA issues and the ACT table load to the top of
    # the main block so their latency overlaps the TileContext entry
    # barrier. Completion sems move with them; consumer waits stay correct
    # (monotone sem-ge).
    blocks = nc.m.functions[0].blocks
    main_bb = blocks[0]
    hoist = []
    for bb in blocks:
        for inst in list(bb.instructions):
            if inst.name.endswith("-PWP"):
                hoist.append(inst)
    hoist += [dma_ci.ins, dma_cf.ins]
    for inst in reversed(hoist):
        for bb in blocks:
            if inst in bb.instructions:
                bb.instructions.remove(inst)
                break
        si = inst.sync_info
        if si is not None and si.on_wait:
            inst.sync_info = mybir.SyncInfo(on_wait=[], on_update=list(si.on_update))
        main_bb.instructions.insert(0, inst)
    _split_multiwaits(nc, k=1)
    return nc


_NC_CACHE = None
LAST_RESULTS = None


def _get_nc():
    global _NC_CACHE
    if _NC_CACHE is None:
        _NC_CACHE = build_bass()
    return _NC_CACHE


def run(pyolos, gboxes_ltrb, labels, trace=False, **spmd_kwargs):
    global LAST_RESULTS
    nc = _get_nc()
    py = np.asarray(pyolos, np.float32).reshape(B, NCH * NANC, HW)
    gbx = np.asarray(gboxes_ltrb, np.float32)
    lbl = np.asarray(labels)

    ct_all, rows = _host_match(gbx, lbl)          # [B*8, 96], [B*8]
    # slot extraction: channels for (img, anchor, cell) of each slot
    plane = (np.asarray(CH_ORDER, np.int64)[None, :] * NANC
             + (rows // HW)[:, None])             # [B*8, 85]
    pos = rows % HW
    img = np.repeat(np.arange(B), NGT)
    slots = py[img[:, None], plane, pos[:, None]]  # [B*8, 85]
    slots[:, 0:3] *= -1.0                          # conf/tx/ty sign flip
    cin_all = np.concatenate([slots, ct_all], axis=1)  # [B*8, KC]

    # pad with 0.0: sigmoid(0)^2 = 0.25 exactly; subtracted from dsq below
    import ml_dtypes
    conf_all = np.zeros((B // BL, CP * CF), ml_dtypes.bfloat16)
    cpc = py[:, 0:NANC, :].reshape(NC, BL * NANC * HW)
    conf_all[:, :BL * NANC * HW] = cpc.astype(ml_dtypes.bfloat16)

    in_maps = []
    for c in range(NC):
        in_maps.append({
            "cin": cin_all.reshape(B, NGT, KC)[c * BL:(c + 1) * BL]
                          .reshape(S, KC),
            "conf": conf_all[c].reshape(CP, CF),
        })
    res = run_bass_kernel_spmd(nc, in_maps, list(range(NC)), trace=trace,
                               **spmd_kwargs)
    LAST_RESULTS = res
    t = np.stack([r["out"] for r in res.results]).astype(np.float64)
    s1 = t[:, :, 0].sum()
    s2 = t[:, :, 1].sum()
    npos = t[:, :, 2].sum()
    s4 = t[:, :, 3].sum()
    s5 = t[:, :, 4].sum()
    s6 = t[:, :, 5].sum()
    npad = CP * CF - BL * NANC * HW
    dsq = t[:, :, 6].sum() - NC * npad * 0.25
    loss = (5.0 * s1 / B
            + (dsq - s2) / B
            + s4 / max(npos, 1.0)
            + s5 / B
            + s6 / B)
    return np.float32(loss)


def kernel(pyolos, gboxes_ltrb, labels):
    return run(pyolos, gboxes_ltrb, labels)
